# revision 1
# baseline (speedup 1.0000x reference)
"""Trainium2 Bass kernel for nn_CDER_64493228917301 (gnn_message_passing).

Reference semantics (GATConv-style, DGL u_dot_v / v_mul_e):
    el  = (e_ft @ W.T).reshape(N, H, F)
    e   = leaky_relu(einsum('ehf,ehf->eh', el[src], el[dst]))
    a   = segment_softmax(e, dst)          # softmax over edges sharing dst
    msg = ft[dst] * a[:, :, None]          # NOTE: uses DESTINATION features
    out = (segment_sum(msg, dst) + bias.reshape(1,H,F)).mean(axis=1)

Key algebraic identity: because the message uses ft[dst] (not ft[src]),
every edge in dst-segment n contributes ft[n] * a_e, and the softmax
weights a_e of one segment sum to 1.  Hence

    segment_sum(msg, dst)[n] = ft[n] * (1 if node n has >=1 in-edge else 0)

exactly (up to f32 rounding of order 1e-7 -- verified global rel err
1.2e-7 vs the jax reference).  The attention logits, the e_ft @ W matmul
and the edge gathers cancel out of the output entirely; the only thing
the edge list contributes is the per-node "has in-edge" indicator.

So the kernel computes, fully on device:

    out[n, f] = (sum_h ft[n, h, f]) * fscale[n] + bias_mean[f]

where fscale[n] = 0.25 * has_in_edge[n] (the 1/H fold is free) and
bias_mean = bias.reshape(H, F).mean(0).  The indicator is produced on
the host during input sharding (a single vectorized scatter over dst --
index preprocessing, like the sharding itself).

Distribution: node-parallel across the 8 NeuronCores.  Each core gets a
12500-node shard (padded to 12544 = 98*128), streams its 6.4 MB of ft
through SBUF in 7 double-buffered 896 KB tiles, does the H-reduction /
scale / bias-add on the vector engine, and writes its 32-float rows
back.  This is purely HBM-bandwidth-bound, which is the target regime.
"""

import numpy as np

N = 100000
H = 4
F = 32
D = H * F            # 128 floats per node in ft
NC = 8               # cores
PER = N // NC        # 12500 nodes per core
P = 128              # SBUF partitions
G = 14               # node-groups per partition per tile
B = 7                # tiles per core
PAD = P * G * B      # 12544 padded nodes per core

_cached = None       # (nc_module,) -- compile once per process


def _build_bass():
    import concourse.tile as tile
    from concourse import bacc, mybir

    nc = bacc.Bacc(
        "TRN2",
        target_bir_lowering=False,
        debug=False,
        enable_asserts=False,
        num_devices=NC,
    )
    f32 = mybir.dt.float32
    ft_in = nc.dram_tensor("ft_in", [PAD, D], f32, kind="ExternalInput").ap()
    fs_in = nc.dram_tensor("fs_in", [PAD], f32, kind="ExternalInput").ap()
    bias_in = nc.dram_tensor("bias_in", [P, F], f32, kind="ExternalInput").ap()
    out = nc.dram_tensor("out", [PAD, F], f32, kind="ExternalOutput").ap()

    # node index n (within the core's shard) = p*(G*B) + b*G + g
    ftv = ft_in.rearrange("(p b g) d -> b p (g d)", p=P, b=B, g=G)   # [B,128,G*D]
    fsv = fs_in.rearrange("(p x) -> p x", p=P)                        # [128, B*G]
    outv = out.rearrange("(p b g) f -> b p (g f)", p=P, b=B, g=G)     # [B,128,G*F]

    with tile.TileContext(nc) as tc:
        with (
            tc.tile_pool(name="const", bufs=1) as cpool,
            tc.tile_pool(name="ft", bufs=3) as ftpool,
            tc.tile_pool(name="tmp", bufs=2) as tpool,
            tc.tile_pool(name="o", bufs=3) as opool,
        ):
            bias_t = cpool.tile([P, F], f32)
            nc.sync.dma_start(bias_t[:], bias_in)
            fs_t = cpool.tile([P, B * G], f32)
            nc.sync.dma_start(fs_t[:], fsv)
            bias_b = bias_t[:].unsqueeze(1).broadcast_to([P, G, F])

            for b in range(B):
                ft_t = ftpool.tile([P, G * D], f32)
                nc.sync.dma_start(ft_t[:], ftv[b])
                ft3 = ft_t[:].rearrange("p (g d) -> p g d", d=D)
                # pairwise head sums: u[p, j, g, f] = ft[.., h=2j, f] + ft[.., h=2j+1, f]
                u_t = tpool.tile([P, 2 * G * F], f32)
                u3 = u_t[:].rearrange("p (j g f) -> p j g f", j=2, g=G)
                in4 = ft_t[:].rearrange("p (g jj f) -> p g jj f", g=G, jj=4)
                nc.vector.tensor_add(
                    u3,
                    in4[:, :, 0:2, :].rearrange("p g j f -> p j g f"),
                    in4[:, :, 2:4, :].rearrange("p g j f -> p j g f"),
                )
                o_t = opool.tile([P, G * F], f32)
                o3 = o_t[:].rearrange("p (g f) -> p g f", f=F)
                nc.vector.tensor_add(o3, u3[:, 0], u3[:, 1])
                fs_b = fs_t[:, b * G : (b + 1) * G].unsqueeze(2).broadcast_to([P, G, F])
                nc.vector.tensor_mul(o3, o3, fs_b)
                nc.vector.tensor_add(o3, o3, bias_b)
                nc.sync.dma_start(outv[b], o_t[:])
    nc.compile()
    return nc


# results of the last device run (for test harness introspection)
LAST_RESULTS = None


def kernel(ft, e_ft, W, bias, src, dst):
    global _cached, LAST_RESULTS
    from concourse import bass_utils

    ft = np.ascontiguousarray(np.asarray(ft, dtype=np.float32)).reshape(N, D)
    bias = np.asarray(bias, dtype=np.float32)
    dst = np.asarray(dst)

    # per-node in-edge indicator, folded with the 1/H of the head mean
    fscale = np.zeros(N, np.float32)
    fscale[dst] = 1.0 / H
    bias_mean = bias.reshape(H, F).mean(axis=0)
    bias_b = np.ascontiguousarray(np.broadcast_to(bias_mean, (P, F)))

    in_maps = []
    for c in range(NC):
        ft_s = np.zeros((PAD, D), np.float32)
        ft_s[:PER] = ft[c * PER : (c + 1) * PER]
        fs_s = np.zeros(PAD, np.float32)
        fs_s[:PER] = fscale[c * PER : (c + 1) * PER]
        in_maps.append({"ft_in": ft_s, "fs_in": fs_s, "bias_in": bias_b})

    if _cached is None:
        _cached = _build_bass()
    nc = _cached

    res = bass_utils.run_bass_kernel_spmd(nc, in_maps, core_ids=list(range(NC)))
    LAST_RESULTS = res
    out = np.empty((N, F), np.float32)
    for c in range(NC):
        out[c * PER : (c + 1) * PER] = res.results[c]["out"][:PER]
    return out
